# revision 12
# baseline (speedup 1.0000x reference)
"""AttentionBlock (GroupNorm + qkv 1x1 + 4-head attention over T=4096 + proj 1x1
+ residual) for b=2, c=256, H=W=64 on 8 NeuronCores.

One (batch, head) pair per core (b*nh = 8 = n_cores). Each core:
  - loads its batch's x [256, 4096] f32 into 8 column-tiles so the GroupNorm
    bn_stats chase the DMA; the reduction chain (Quake rsqrt on the DVE, no
    Sqrt table load) produces per-channel a_ch/b_ch
  - folds GroupNorm into the qkv weights on device (wq' = wq * a_ch; biases
    get + wX^T b_ch), so there is no xn activation pass over [256, 4096] and
    the q/k/v matmuls consume x directly as f32r (fp22). The v-channel
    constant (wv^T b_ch + bv) commutes through softmax and proj, so b_ch
    ships to the host and folds into the gather.
  - computes q, k [128, 4096] bf16 zero-padded to K=128 (full-array
    contraction keeps the HAM clock gate warm at 2.4 GHz; K=64 row-tiled
    variants run cold/serial) and vT [4096, 65] fp8e4m3 with a ones column
    (row 64 of the h accumulator is the softmax denominator for free)
  - attention with transposed scores, 2 key-blocks per PSUM slot, 3 slots
    rotating: sT = k_blk.T @ q -> exp -> h += vT.T @ p. exp is split
    between the Scalar engine (AF.Exp -> fp8, table set preloaded behind the
    x DMA by a dummy activation) and the Vector engine (one-op int8
    Schraudolph: int8(A*s + B) bitcast to fp8e4m3) by a greedy ns balancer,
    so every pv matmul is an fp8 DoubleRow pair. pv matmuls are emitted
    PV_LAG slots behind their scores so the in-order PE queue never stalls
    waiting on exp.
  - unnormalized proj partial [256, 4096] = wpT.T @ h streamed out per
    512-column chunk, plus the per-column softmax denominators

Host gather: out[b] = x[b] + proj_b + sum_h(partial_h / rowsum_h
+ wp_h @ (wv_h^T b_ch + bv_h)). The rowsum division and the v-bias term
commute with the channel contraction, so they fold into the per-element
combine the gather already does.

Requires ~1-wait-per-instruction BIR legalization for this container's
walrus (see _legalize_bir_waits).
"""

import sys
import types

import numpy as np
import ml_dtypes

# ---------------------------------------------------------------------------
# Environment shims (axon container): NTFF profile hook + no artifact upload.
# ---------------------------------------------------------------------------


def _install_shims():
    if "antenv.axon_hooks" not in sys.modules:
        mod = types.ModuleType("antenv.axon_hooks")
        _hook = [None]
        mod.set_axon_ntff_profile_hook = lambda h: _hook.__setitem__(0, h)
        mod.get_axon_ntff_profile_hook = lambda: _hook[0]
        sys.modules["antenv.axon_hooks"] = mod
        try:
            import antenv

            antenv.axon_hooks = mod
            from trn_agent_boot.trn_boot import _ntff_profile_via_ctypes

            mod.set_axon_ntff_profile_hook(
                _ntff_profile_via_ctypes("/opt/axon/libaxon_pjrt.so")
            )
        except Exception:
            pass
    import concourse.bass_utils as bass_utils

    bass_utils.upload_artifacts = lambda d: d


_install_shims()

import concourse.bass as bass
import concourse.mybir as mybir
import concourse.tile as tile
from concourse.bass_utils import run_bass_kernel_spmd

F32 = mybir.dt.float32
F32R = mybir.dt.float32r
BF16 = mybir.dt.bfloat16
I32 = mybir.dt.int32
I16 = mybir.dt.int16
I8 = mybir.dt.int8
FP8 = mybir.dt.float8e4
AF = mybir.ActivationFunctionType
ALU = mybir.AluOpType

B, C, HW, T = 2, 256, 64, 4096
NH, CH = 4, 64  # heads, channels per head
NG, GS = 32, 8  # groups, channels per group
EPS = 1e-5
N_CORES = 8
TC = 512  # t-chunk width
N_TCHUNKS = T // TC  # 8
N_SBLK = T // 128  # 32 key blocks of 128
N_PAIR = N_SBLK // 2  # 16 row-tiled pairs / exp slots per t-chunk

# Schraudolph fast-exp emitting fp8e4m3 bits via int8:
#   p = exp(0.125*s);  bits8 = round(8*(0.125*s*log2(e) + 7 - 0.0437))
LOG2E = 1.4426950408889634
EXP_SC = 0.125
A8 = 8.0 * EXP_SC * LOG2E
B8 = 8.0 * (7.0 - 0.0437)

# ---------------------------------------------------------------------------
# BIR wait legalization: this container's walrus accepts at most ONE sync wait
# per instruction (two for EventSemaphore); hoist excess waits onto inserted
# EventSemaphores on the same engine.
# ---------------------------------------------------------------------------


def _legalize_bir_waits(bir_bytes: bytes) -> bytes:
    import json

    m = json.loads(bir_bytes)
    changed = False
    for fn in m["functions"]:
        for blk in fn["blocks"]:
            new_insts = []
            for inst in blk["instructions"]:
                si = inst.get("sync_info")
                waits = list(si.get("on_wait") or []) if si else []
                cap = 2 if inst.get("opcode") == "EventSemaphore" else 1
                if len(waits) > cap:
                    changed = True
                    keep = waits[-cap:]
                    extra = waits[:-cap]
                    idx = 0
                    while extra:
                        chunk, extra = extra[:2], extra[2:]
                        es = {
                            "name": f"{inst['name']}_ws{idx}",
                            "engine": inst["engine"],
                            "opcode": "EventSemaphore",
                            "ins": [],
                            "outs": [],
                            "sync_info": {"on_wait": chunk, "on_update": []},
                        }
                        if "debug" in inst:
                            es["debug"] = inst["debug"]
                        new_insts.append(es)
                        idx += 1
                    si["on_wait"] = keep
                new_insts.append(inst)
            blk["instructions"] = new_insts
    return json.dumps(m).encode() if changed else bir_bytes


# ---------------------------------------------------------------------------
# Device program (identical on all 8 cores; inputs differ per core)
# ---------------------------------------------------------------------------


def build_nc():
    nc = bass.Bass()

    x_in = nc.dram_tensor("x", [C, T], F32R, kind="ExternalInput")
    wqT_in = nc.dram_tensor("wqT", [C, CH], F32, kind="ExternalInput")
    wkT_in = nc.dram_tensor("wkT", [C, CH], F32, kind="ExternalInput")
    wvT_in = nc.dram_tensor("wvT", [C, CH], F32, kind="ExternalInput")
    bq_in = nc.dram_tensor("bq", [CH, 1], F32, kind="ExternalInput")
    bk_in = nc.dram_tensor("bk", [CH, 1], F32, kind="ExternalInput")
    # proj weights zero-padded to K=128 (rows CH..127 are zero)
    wpT_in = nc.dram_tensor("wpT", [128, C], BF16, kind="ExternalInput")
    oh_in = nc.dram_tensor("oh", [128, 16], F32, kind="ExternalInput")
    ohT_in = nc.dram_tensor("ohT", [16, 128], F32, kind="ExternalInput")
    gnw_in = nc.dram_tensor("gnw", [C, 1], F32, kind="ExternalInput")
    gnb_in = nc.dram_tensor("gnb", [C, 1], F32, kind="ExternalInput")
    out = nc.dram_tensor("out", [C, T], F32, kind="ExternalOutput")
    # softmax denominators (host divides during the gather)
    rsum_out = nc.dram_tensor("rsum", [1, T], BF16, kind="ExternalOutput")
    # GroupNorm b_ch per channel (host folds wv^T b_ch + bv into the gather)
    bch_out = nc.dram_tensor("bch", [2, 128], F32, kind="ExternalOutput")

    with tile.TileContext(nc) as tc:
        with (
            tc.tile_pool(name="const", bufs=1) as const,
            tc.tile_pool(name="xp", bufs=1) as xp,
            tc.tile_pool(name="qk", bufs=1) as qkp,
            tc.tile_pool(name="gn", bufs=2) as gn,
            tc.tile_pool(name="pp", bufs=5) as ppool,
            tc.tile_pool(name="pip", bufs=5) as pip,
            tc.tile_pool(name="op", bufs=4) as op,
            tc.tile_pool(name="ps", bufs=3, space="PSUM") as ps,
            tc.tile_pool(name="ph", bufs=2, space="PSUM") as ph,
        ):
            # ---- x load: 8 column-tiles so bn_stats chases the DMA; issued
            # from the (idle) tensor-engine DMA queue so the small const DMAs
            # on the sync queue don't serialize behind it ----
            x_cols = []  # 8 tiles of [128, 1024]: [half][quarter]
            for i in range(2):
                for quarter in range(4):
                    x_t = xp.tile(
                        [128, 1024], F32R, tag=f"x{i}_{quarter}",
                        name=f"x{i}_{quarter}",
                    )
                    qsl = slice(quarter * 1024, (quarter + 1) * 1024)
                    # half 0 on the sync HWDGE queue, half 1 via gpsimd SWDGE:
                    # two parallel DMA streams halve the x load time
                    dma_eng = nc.sync if i == 0 else nc.gpsimd
                    dma_eng.dma_start(
                        out=x_t, in_=x_in[i * 128 : (i + 1) * 128, qsl]
                    )
                    x_cols.append(x_t)

            def x_ap(ki, lo, hi):
                """view of x[ki*128:(ki+1)*128, lo:hi] (within one col-tile)"""
                t = x_cols[ki * 4 + lo // 1024]
                base = (lo // 1024) * 1024
                return t[:, lo - base : hi - base]

            # ---- load constants/weights (scalar-issued so they don't queue
            # behind the x chunks on the sync engine) ----
            def load_const(name, src, shape, dtype):
                t = const.tile(shape, dtype, tag=name)
                nc.scalar.dma_start(out=t, in_=src[:, :])
                return t

            wq_raw = [
                const.tile([128, CH], F32, tag=f"wqr{i}", name=f"wqr{i}")
                for i in range(2)
            ]
            wk_raw = [
                const.tile([128, CH], F32, tag=f"wkr{i}", name=f"wkr{i}")
                for i in range(2)
            ]
            wv_raw = [
                const.tile([128, CH], F32, tag=f"wvr{i}", name=f"wvr{i}")
                for i in range(2)
            ]
            for i in range(2):
                ksl = slice(i * 128, (i + 1) * 128)
                nc.scalar.dma_start(out=wq_raw[i], in_=wqT_in[ksl, :])
                nc.scalar.dma_start(out=wk_raw[i], in_=wkT_in[ksl, :])
                nc.scalar.dma_start(out=wv_raw[i], in_=wvT_in[ksl, :])

            bq_sb = load_const("bq", bq_in, [CH, 1], F32)
            bk_sb = load_const("bk", bk_in, [CH, 1], F32)
            wpT_sb = load_const("wpT", wpT_in, [128, C], BF16)
            oh_sb = load_const("oh", oh_in, [128, 16], F32)
            ohT_sb = load_const("ohT", ohT_in, [16, 128], F32)

            gnw_t = [
                const.tile([128, 1], F32, tag=f"gnw{i}", name=f"gnw{i}")
                for i in range(2)
            ]
            gnb_t = [
                const.tile([128, 1], F32, tag=f"gnb{i}", name=f"gnb{i}")
                for i in range(2)
            ]
            for i in range(2):
                nc.scalar.dma_start(out=gnw_t[i], in_=gnw_in[i * 128 : (i + 1) * 128, :])
                nc.scalar.dma_start(out=gnb_t[i], in_=gnb_in[i * 128 : (i + 1) * 128, :])

            # preload the Exp table set while the x DMA streams
            warm = gn.tile([1, 1], F32, tag="warm")
            nc.vector.memset(warm, 0.0)
            warm2 = gn.tile([1, 1], F32, tag="warm2")
            nc.scalar.activation(out=warm2, in_=warm, func=AF.Exp)

            # persistent double-buffered hu tiles: rows CH+1..127 zeroed once
            hu_t = []
            for i in range(2):
                t = const.tile([128, TC], BF16, tag=f"hu{i}", name=f"hu{i}")
                nc.gpsimd.memset(t[CH:128, :], 0.0)
                hu_t.append(t)

            # ---- GroupNorm stats -> per-channel a_ch (scale), b_ch (shift).
            # All 16 bn_stats are emitted first (the DVE queue is FIFO, so
            # half-1's stats must not sit behind half-0's reduction chain),
            # then the two halves' chains run interleaved so the per-op
            # semaphore latencies overlap. ----
            stats_t = []
            for i in range(2):
                stats = gn.tile([128, 8, 6], F32, tag=f"stats{i}", name=f"st{i}")
                for j in range(8):
                    xf = x_cols[i * 4 + j // 2].bitcast(F32)
                    wsl = slice((j % 2) * 512, (j % 2) * 512 + 512)
                    nc.vector.bn_stats(out=stats[:, j, :], in_=xf[:, wsl])
                stats_t.append(stats)

            def chain_step(fn):
                for i in range(2):
                    fn(i)

            st = [dict() for _ in range(2)]

            def mk(i, nm, shape=(16, 1), dtype=F32):
                t = gn.tile(list(shape), dtype, tag=f"{nm}{i}", name=f"{nm}{i}")
                st[i][nm] = t
                return t

            chain_step(lambda i: nc.vector.bn_aggr(
                out=mk(i, "mv", (128, 2)), in_=stats_t[i]))
            # mq = [mean, var + mean^2] per channel
            chain_step(lambda i: nc.vector.tensor_copy(
                out=mk(i, "mq", (128, 2))[:, 0:1], in_=st[i]["mv"][:, 0:1]))
            chain_step(lambda i: nc.vector.scalar_tensor_tensor(
                out=st[i]["mq"][:, 1:2], in0=st[i]["mv"][:, 0:1],
                scalar=st[i]["mv"][:, 0:1], in1=st[i]["mv"][:, 1:2],
                op0=ALU.mult, op1=ALU.add))

            # group reduce: [16, 2] = oh.T @ mq   (oh entries are 1/8)
            ps_g = ph.tile([16, 4], F32, tag="ph")
            for i in range(2):
                nc.tensor.matmul(
                    ps_g[:, 2 * i : 2 * i + 2], lhsT=oh_sb, rhs=st[i]["mq"],
                    start=True, stop=True,
                )
            chain_step(lambda i: nc.vector.tensor_copy(
                out=mk(i, "gstats", (16, 2)), in_=ps_g[:, 2 * i : 2 * i + 2]))

            # gve = E[x^2] - mean^2 + eps
            chain_step(lambda i: nc.vector.scalar_tensor_tensor(
                out=mk(i, "gve"), in0=st[i]["gstats"][:, 0:1],
                scalar=st[i]["gstats"][:, 0:1], in1=st[i]["gstats"][:, 1:2],
                op0=ALU.mult, op1=ALU.subtract))
            chain_step(lambda i: nc.vector.tensor_scalar(
                out=st[i]["gve"], in0=st[i]["gve"], scalar1=-1.0, scalar2=EPS,
                op0=ALU.mult, op1=ALU.add))
            # Quake rsqrt + 2 Newton iterations (avoids the Sqrt table set)
            chain_step(lambda i: nc.vector.tensor_scalar(
                out=mk(i, "iv", dtype=I32), in0=st[i]["gve"].bitcast(I32),
                scalar1=1, scalar2=None, op0=ALU.logical_shift_right))
            chain_step(lambda i: nc.vector.tensor_scalar(
                out=st[i]["iv"], in0=st[i]["iv"], scalar1=-1,
                scalar2=0x5F3759DF, op0=ALU.mult, op1=ALU.add))
            chain_step(lambda i: nc.vector.tensor_scalar(
                out=mk(i, "hv"), in0=st[i]["gve"], scalar1=0.5, scalar2=None,
                op0=ALU.mult))
            for i in range(2):
                st[i]["cur"] = st[i]["iv"].bitcast(F32)
                mk(i, "yy")
                mk(i, "t2")
                mk(i, "grstd")
            for it in range(2):
                chain_step(lambda i: nc.vector.tensor_tensor(
                    out=st[i]["yy"], in0=st[i]["cur"], in1=st[i]["cur"],
                    op=ALU.mult))
                chain_step(lambda i: nc.vector.tensor_tensor(
                    out=st[i]["yy"], in0=st[i]["yy"], in1=st[i]["hv"],
                    op=ALU.mult))
                chain_step(lambda i: nc.vector.tensor_scalar(
                    out=st[i]["yy"], in0=st[i]["yy"], scalar1=-1.0, scalar2=1.5,
                    op0=ALU.mult, op1=ALU.add))

                def newt(i, it=it):
                    dst = st[i]["grstd"] if it == 1 else st[i]["t2"]
                    nc.vector.tensor_tensor(
                        out=dst, in0=st[i]["cur"], in1=st[i]["yy"], op=ALU.mult
                    )
                    st[i]["cur"] = dst
                chain_step(newt)

            chain_step(lambda i: nc.vector.tensor_copy(
                out=mk(i, "gmr", (16, 2))[:, 0:1], in_=st[i]["gstats"][:, 0:1]))
            chain_step(lambda i: nc.vector.tensor_copy(
                out=st[i]["gmr"][:, 1:2], in_=st[i]["grstd"]))

            # broadcast back to channels: [128, 2] = ohT.T @ gmr
            ps_bc = ph.tile([128, 4], F32, tag="ph")
            for i in range(2):
                nc.tensor.matmul(
                    ps_bc[:, 2 * i : 2 * i + 2], lhsT=ohT_sb, rhs=st[i]["gmr"],
                    start=True, stop=True,
                )
            chain_step(lambda i: nc.vector.tensor_tensor(
                out=mk(i, "a_ch", (128, 1)), in0=ps_bc[:, 2 * i + 1 : 2 * i + 2],
                in1=gnw_t[i], op=ALU.mult))
            chain_step(lambda i: nc.vector.tensor_tensor(
                out=mk(i, "t1", (128, 1)), in0=ps_bc[:, 2 * i : 2 * i + 1],
                in1=st[i]["a_ch"], op=ALU.mult))
            chain_step(lambda i: nc.vector.tensor_tensor(
                out=mk(i, "b_ch", (128, 1)), in0=gnb_t[i], in1=st[i]["t1"],
                op=ALU.subtract))
            for i in range(2):
                nc.sync.dma_start(out=bch_out[i : i + 1, :], in_=st[i]["b_ch"])
            a_ch = [(st[i]["a_ch"], st[i]["b_ch"]) for i in range(2)]

            # ---- fold GN into the qkv weights/biases ----
            wqf = [
                qkp.tile([128, CH], F32R, tag=f"wqf{i}", name=f"wqf{i}")
                for i in range(2)
            ]
            wkf = [
                qkp.tile([128, CH], F32R, tag=f"wkf{i}", name=f"wkf{i}")
                for i in range(2)
            ]
            wvf = [
                qkp.tile([128, CH], F32R, tag=f"wvf{i}", name=f"wvf{i}")
                for i in range(2)
            ]
            for i in range(2):
                nc.scalar.activation(
                    out=wqf[i], in_=wq_raw[i], func=AF.Identity, scale=a_ch[i][0]
                )
                nc.scalar.activation(
                    out=wkf[i], in_=wk_raw[i], func=AF.Identity, scale=a_ch[i][0]
                )
                nc.scalar.activation(
                    out=wvf[i], in_=wv_raw[i], func=AF.Identity, scale=a_ch[i][0]
                )

            # bias' = b + wX_raw^T b_ch  (tiny N=1 matmuls, f32)
            ps_bias = ph.tile([CH, 2], F32, tag="ph")
            for col, wr in ((0, wq_raw), (1, wk_raw)):
                for ki in range(2):
                    nc.tensor.matmul(
                        ps_bias[:, col : col + 1],
                        lhsT=wr[ki],
                        rhs=a_ch[ki][1],
                        start=(ki == 0),
                        stop=(ki == 1),
                    )
            bqf = gn.tile([CH, 1], F32, tag="bqf")
            bkf = gn.tile([CH, 1], F32, tag="bkf")
            nc.vector.tensor_tensor(
                out=bqf, in0=ps_bias[:, 0:1], in1=bq_sb, op=ALU.add
            )
            nc.vector.tensor_tensor(
                out=bkf, in0=ps_bias[:, 1:2], in1=bk_sb, op=ALU.add
            )

            # ---- q/k/v targets ----
            # q, k zero-padded to K=128: the full-array contraction keeps the
            # HAM clock gate warm at 2.4 GHz (K=64 variants run cold/serial)
            q_sb = qkp.tile([128, T], BF16, tag="q")
            k_sb = qkp.tile([128, T], BF16, tag="k")
            nc.gpsimd.memset(q_sb[CH:128, :], 0.0)
            nc.gpsimd.memset(k_sb[CH:128, :], 0.0)
            vT = qkp.tile([128, N_SBLK * 80], FP8, tag="vT")
            nc.gpsimd.memset(vT, 1.0)
            vT_view = vT.rearrange("p (b c) -> p b c", c=80)

            ring = [0]  # rotation counter (pool handles the actual slots)

            def ring_slice():
                r = ring[0] % 3
                ring[0] += 1
                return r, ps.tile([128, 1024], F32, tag="ps", name="ps_s")

            def emit_qk_chunk(dst, w, bias, n):
                _, psq = ring_slice()
                psq = psq[0:CH, :]
                for nj in range(2):
                    sl = slice(nj * 512, (nj + 1) * 512)
                    lo = n * 1024 + nj * 512
                    for ki in range(2):
                        nc.tensor.matmul(
                            psq[:, sl],
                            lhsT=w[ki],
                            rhs=x_ap(ki, lo, lo + 512),
                            start=(ki == 0),
                            stop=(ki == 1),
                        )
                with nc.allow_low_precision(reason="bf16 q/k"):
                    nc.scalar.activation(
                        out=dst[0:CH, n * 1024 : (n + 1) * 1024],
                        in_=psq,
                        func=AF.Identity,
                        bias=bias,
                    )

            def emit_vt_chunk(pblk):
                psv = ph.tile([128, 512], F32, tag="ph", name=f"psv{pblk}")
                for j in range(8):
                    sblk = pblk * 8 + j
                    sl = slice(j * 64, (j + 1) * 64)
                    for ki in range(2):
                        nc.tensor.matmul(
                            psv[:, sl],
                            lhsT=x_ap(ki, sblk * 128, (sblk + 1) * 128),
                            rhs=wvf[ki],
                            start=(ki == 0),
                            stop=(ki == 1),
                        )
                with nc.allow_low_precision(reason="fp8 v"):
                    nc.scalar.copy(
                        out=vT_view[:, pblk * 8 : (pblk + 1) * 8, 0:64],
                        in_=psv.rearrange("p (b c) -> p b c", c=64),
                    )

            def emit_qkv_step(c):
                emit_qk_chunk(k_sb, wkf, bkf, c)
                if c == 0:
                    emit_qk_chunk(q_sb, wqf, bqf, 0)

            emit_qkv_step(0)

            # ---- attention + proj, streamed per t-chunk ----
            PV_LAG = 3  # pv(j) is emitted
            # after scores(j+PV_LAG) so the in-order PE queue never stalls
            # waiting for exp(j)

            # greedy engine balance for exp work (ns accounting); t-chunk 0
            # pre-charges the scalar engine for the q/k/v PSUM->SBUF copies
            ebal = {"S": 0.0, "V": 0.0}

            def exp_engine(width):
                cost_s = 352 + width
                cost_v = (120 + width) * 1.25
                if ebal["S"] + cost_s <= ebal["V"] + cost_v:
                    ebal["S"] += cost_s
                    return "S"
                ebal["V"] += cost_v
                return "V"

            def emit_exp(slots):
                """slots: list of (j, ps_s tile), len 1 (no cross-tile merge)."""
                assert len(slots) == 1
                width = 1024
                src = slots[0][1]
                eng = exp_engine(width)
                if eng == "V":
                    pi_t = pip.tile([128, width], I8, tag=f"pi{len(slots)}",
                                    name="pi")
                    with nc.allow_low_precision(reason="fast exp"):
                        nc.vector.tensor_scalar(
                            out=pi_t, in0=src, scalar1=A8, scalar2=B8,
                            op0=ALU.mult, op1=ALU.add,
                        )
                    pb = pi_t.bitcast(FP8)
                else:
                    pb = ppool.tile([128, width], FP8, tag=f"p{len(slots)}",
                                    name="p")
                    with nc.allow_low_precision(reason="fp8 p"):
                        nc.scalar.activation(
                            out=pb, in_=src, func=AF.Exp, scale=EXP_SC,
                        )
                return [
                    (j, pb[:, i * 1024 : (i + 1) * 1024])
                    for i, (j, _) in enumerate(slots)
                ]

            T0_CHARGE = 11000.0  # tchunk-0 scalar precharge (q/k/v copies)

            def body(tci):
                tsl = slice(tci * TC, (tci + 1) * TC)
                if tci == 0:
                    ebal["S"] += T0_CHARGE
                ps_h = ph.tile([65, TC], F32, tag="ph", name=f"ps_h{tci}")
                pending = []  # pv operands awaiting emission
                unexp = []  # score slots awaiting exp

                def emit_pv(j, pb):
                    pr = pb.rearrange("p (b c) -> p b c", c=TC)
                    nc.tensor.matmul(
                        ps_h,
                        lhsT=vT_view[:, 2 * j : 2 * j + 2, 0:65],
                        rhs=pr[:, 0:2, :],
                        start=(j == 0),
                        stop=(j == N_PAIR - 1),
                        perf_mode=mybir.MatmulPerfMode.DoubleRow,
                    )

                def flush_exp():
                    if unexp:
                        pending.extend(emit_exp(unexp))
                        unexp.clear()

                for j in range(N_PAIR):
                    if tci == 0 and j in (4, 8, 12):
                        flush_exp()
                        emit_qkv_step(j // 4)
                        if j == 4:
                            for qc in range(1, 4):
                                emit_qk_chunk(q_sb, wqf, bqf, qc)
                    if tci == 0 and j in (1, 5, 9, 13):
                        # v chunks trail the k/q chunks so the first scores
                        # (and HAM warm-up) start as early as possible
                        emit_vt_chunk(j // 4)
                    r, ps_s = ring_slice()
                    for jj in range(2):
                        sblk = 2 * j + jj
                        nc.tensor.matmul(
                            ps_s[:, jj * 512 : (jj + 1) * 512],
                            lhsT=k_sb[:, sblk * 128 : (sblk + 1) * 128],
                            rhs=q_sb[:, tsl],
                            start=True,
                            stop=True,
                        )
                    unexp.append((j, ps_s))
                    flush_exp()
                    while len(pending) > PV_LAG:
                        emit_pv(*pending.pop(0))
                flush_exp()
                for item in pending:
                    emit_pv(*item)
                return ps_h

            def epilogue(tci, ps_h):
                tsl = slice(tci * TC, (tci + 1) * TC)
                # ship the softmax denominators; the division commutes with
                # the proj channel-contraction and the host's gather applies it
                hu = hu_t[tci % 2]
                with nc.allow_low_precision(reason="bf16 h"):
                    nc.vector.tensor_copy(out=hu[0:65, :], in_=ps_h[0:65, :])
                nc.sync.dma_start(out=rsum_out[0:1, tsl], in_=hu[64:65, :])
                for mi in range(2):
                    pp_ps = ph.tile([128, TC], F32, tag="ph", name=f"pp{tci}_{mi}")
                    nc.tensor.matmul(
                        pp_ps,
                        lhsT=wpT_sb[:, mi * 128 : (mi + 1) * 128],
                        rhs=hu,
                        start=True,
                        stop=True,
                    )
                    o_t = op.tile([128, TC], F32, tag="o", name=f"o{tci}_{mi}")
                    if mi == 0:
                        nc.scalar.copy(out=o_t, in_=pp_ps)
                    else:
                        nc.vector.tensor_copy(out=o_t, in_=pp_ps)
                    nc.sync.dma_start(
                        out=out[mi * 128 : (mi + 1) * 128, tsl], in_=o_t
                    )

            # software pipeline: emit chunk i+1's matmuls before chunk i's
            # epilogue so the PE never stalls on the epilogue path
            prev = None
            for tci in range(N_TCHUNKS):
                ps_h = body(tci)
                if prev is not None:
                    epilogue(tci - 1, prev)
                prev = ps_h
            epilogue(N_TCHUNKS - 1, prev)

    # wrap to_json_bytes with the wait legalization
    orig = nc.to_json_bytes
    nc.to_json_bytes = lambda *a, **k: _legalize_bir_waits(orig(*a, **k))
    return nc


_NC = None


def _get_nc():
    global _NC
    if _NC is None:
        _NC = build_nc()
    return _NC


def _make_in_maps(inputs):
    x = np.asarray(inputs["x"], dtype=np.float32)
    gn_w = np.asarray(inputs["gn_w"], dtype=np.float32)
    gn_b = np.asarray(inputs["gn_b"], dtype=np.float32)
    qkv_w = np.asarray(inputs["qkv_w"], dtype=np.float32)
    qkv_b = np.asarray(inputs["qkv_b"], dtype=np.float32)
    proj_w = np.asarray(inputs["proj_w"], dtype=np.float32)

    xs = x.reshape(B, C, T)
    oh = np.kron(np.eye(16, dtype=np.float32), np.full((8, 1), 0.125, np.float32))
    ohT = np.ascontiguousarray(oh.T) * 8.0  # plain one-hot [16, 128]
    gnw = gn_w.reshape(C, 1)
    gnb = gn_b.reshape(C, 1)

    in_maps = []
    for core in range(N_CORES):
        b, h = divmod(core, NH)
        # reference reshapes (b, 3c, T) -> (b*nh, 3*ch, T) then splits dim 1,
        # so head h takes qkv rows [3*ch*h : 3*ch*(h+1)] as [q | k | v]
        base = 3 * CH * h
        qsl = slice(base, base + CH)
        ksl = slice(base + CH, base + 2 * CH)
        vsl = slice(base + 2 * CH, base + 3 * CH)
        wqT = np.ascontiguousarray(qkv_w[qsl, :].T).astype(np.float32)
        wkT = np.ascontiguousarray(qkv_w[ksl, :].T).astype(np.float32)
        wvT = np.ascontiguousarray(qkv_w[vsl, :].T).astype(np.float32)
        bq = qkv_b[qsl].reshape(CH, 1).astype(np.float32)
        bk = qkv_b[ksl].reshape(CH, 1).astype(np.float32)
        # after attention, h.reshape(b, c, T) stacks heads along channels:
        # head h occupies channels [ch*h : ch*(h+1)]; padded to K=128 rows
        wpT = np.zeros((128, C), ml_dtypes.bfloat16)
        wpT[0:CH] = proj_w[:, h * CH : (h + 1) * CH].T.astype(ml_dtypes.bfloat16)
        in_maps.append(
            {
                "x": np.ascontiguousarray(xs[b]),
                "wqT": wqT,
                "wkT": wkT,
                "wvT": wvT,
                "bq": bq,
                "bk": bk,
                "wpT": wpT,
                "oh": oh,
                "ohT": ohT,
                "gnw": gnw,
                "gnb": gnb,
            }
        )
    return in_maps


def _combine(inputs, results):
    x = np.asarray(inputs["x"], dtype=np.float32)
    proj_b = np.asarray(inputs["proj_b"], dtype=np.float32)
    qkv_b = np.asarray(inputs["qkv_b"], dtype=np.float32)
    qkv_w = np.asarray(inputs["qkv_w"], dtype=np.float32)
    proj_w = np.asarray(inputs["proj_w"], dtype=np.float32)
    xs = x.reshape(B, C, T)
    out = np.empty((B, C, T), np.float32)
    for b in range(B):
        acc = xs[b] + proj_b[:, None]
        b_ch = results[b * NH]["bch"].reshape(C)
        for h in range(NH):
            r = results[b * NH + h]
            # v's effective bias (wv^T b_ch + bv) contributes a constant per
            # channel to h after the rowsum division; it commutes with proj
            vsl = slice(3 * CH * h + 2 * CH, 3 * CH * (h + 1))
            bv_eff = qkv_w[vsl, :] @ b_ch + qkv_b[vsl]
            wpbv = proj_w[:, h * CH : (h + 1) * CH] @ bv_eff
            acc = (
                acc
                + r["out"] * (1.0 / r["rsum"][0].astype(np.float32))[None, :]
                + wpbv[:, None]
            )
        out[b] = acc
    return out.reshape(B, C, HW, HW)


def _run(inputs, trace=False, trace_kwargs=None):
    nc = _get_nc()
    in_maps = _make_in_maps(inputs)
    res = run_bass_kernel_spmd(
        nc,
        in_maps,
        core_ids=list(range(N_CORES)),
        trace=trace,
        **(trace_kwargs or {}),
    )
    return _combine(inputs, res.results), res


def kernel(**inputs) -> np.ndarray:
    out, _ = _run(inputs, trace=False)
    return out


# revision 13
# speedup vs baseline: 1.0427x; 1.0427x over previous
"""AttentionBlock (GroupNorm + qkv 1x1 + 4-head attention over T=4096 + proj 1x1
+ residual) for b=2, c=256, H=W=64 on 8 NeuronCores.

One (batch, head) pair per core (b*nh = 8 = n_cores). Each core:
  - loads its batch's x [256, 4096] f32 into 8 column-tiles so the GroupNorm
    bn_stats chase the DMA; the reduction chain (Quake rsqrt on the DVE, no
    Sqrt table load) produces per-channel a_ch/b_ch
  - folds GroupNorm into the qkv weights on device (wq' = wq * a_ch; biases
    get + wX^T b_ch), so there is no xn activation pass over [256, 4096] and
    the q/k/v matmuls consume x directly as f32r (fp22). The v-channel
    constant (wv^T b_ch + bv) commutes through softmax and proj, so b_ch
    ships to the host and folds into the gather.
  - computes q, k [128, 4096] bf16 zero-padded to K=128 (full-array
    contraction keeps the HAM clock gate warm at 2.4 GHz; K=64 row-tiled
    variants run cold/serial) and vT [4096, 65] fp8e4m3 with a ones column
    (row 64 of the h accumulator is the softmax denominator for free)
  - attention with transposed scores, 2 key-blocks per PSUM slot, 3 slots
    rotating: sT = k_blk.T @ q -> exp -> h += vT.T @ p. exp is split
    between the Scalar engine (AF.Exp -> fp8, table set preloaded behind the
    x DMA by a dummy activation) and the Vector engine (one-op int8
    Schraudolph: int8(A*s + B) bitcast to fp8e4m3) by a greedy ns balancer,
    so every pv matmul is an fp8 DoubleRow pair. pv matmuls are emitted
    PV_LAG slots behind their scores so the in-order PE queue never stalls
    waiting on exp.
  - unnormalized proj partial [256, 4096] = wpT.T @ h streamed out per
    512-column chunk, plus the per-column softmax denominators

Host gather: out[b] = x[b] + proj_b + sum_h(partial_h / rowsum_h
+ wp_h @ (wv_h^T b_ch + bv_h)). The rowsum division and the v-bias term
commute with the channel contraction, so they fold into the per-element
combine the gather already does.

Requires ~1-wait-per-instruction BIR legalization for this container's
walrus (see _legalize_bir_waits).
"""

import sys
import types

import numpy as np
import ml_dtypes

# ---------------------------------------------------------------------------
# Environment shims (axon container): NTFF profile hook + no artifact upload.
# ---------------------------------------------------------------------------


def _install_shims():
    if "antenv.axon_hooks" not in sys.modules:
        mod = types.ModuleType("antenv.axon_hooks")
        _hook = [None]
        mod.set_axon_ntff_profile_hook = lambda h: _hook.__setitem__(0, h)
        mod.get_axon_ntff_profile_hook = lambda: _hook[0]
        sys.modules["antenv.axon_hooks"] = mod
        try:
            import antenv

            antenv.axon_hooks = mod
            from trn_agent_boot.trn_boot import _ntff_profile_via_ctypes

            mod.set_axon_ntff_profile_hook(
                _ntff_profile_via_ctypes("/opt/axon/libaxon_pjrt.so")
            )
        except Exception:
            pass
    import concourse.bass_utils as bass_utils

    bass_utils.upload_artifacts = lambda d: d


_install_shims()

import concourse.bass as bass
import concourse.mybir as mybir
import concourse.tile as tile
from concourse.bass_utils import run_bass_kernel_spmd

F32 = mybir.dt.float32
F32R = mybir.dt.float32r
BF16 = mybir.dt.bfloat16
I32 = mybir.dt.int32
I16 = mybir.dt.int16
I8 = mybir.dt.int8
FP8 = mybir.dt.float8e4
AF = mybir.ActivationFunctionType
ALU = mybir.AluOpType

B, C, HW, T = 2, 256, 64, 4096
NH, CH = 4, 64  # heads, channels per head
NG, GS = 32, 8  # groups, channels per group
EPS = 1e-5
N_CORES = 8
TC = 512  # t-chunk width
N_TCHUNKS = T // TC  # 8
N_SBLK = T // 128  # 32 key blocks of 128
N_PAIR = N_SBLK // 2  # 16 row-tiled pairs / exp slots per t-chunk

# Schraudolph fast-exp emitting fp8e4m3 bits via int8:
#   p = exp(0.125*s);  bits8 = round(8*(0.125*s*log2(e) + 7 - 0.0437))
LOG2E = 1.4426950408889634
EXP_SC = 0.125
A8 = 8.0 * EXP_SC * LOG2E
B8 = 8.0 * (7.0 - 0.0437)

# ---------------------------------------------------------------------------
# BIR wait legalization: this container's walrus accepts at most ONE sync wait
# per instruction (two for EventSemaphore); hoist excess waits onto inserted
# EventSemaphores on the same engine.
# ---------------------------------------------------------------------------


def _legalize_bir_waits(bir_bytes: bytes) -> bytes:
    import json

    m = json.loads(bir_bytes)
    changed = False
    for fn in m["functions"]:
        for blk in fn["blocks"]:
            new_insts = []
            for inst in blk["instructions"]:
                si = inst.get("sync_info")
                waits = list(si.get("on_wait") or []) if si else []
                cap = 2 if inst.get("opcode") == "EventSemaphore" else 1
                if len(waits) > cap:
                    changed = True
                    keep = waits[-cap:]
                    extra = waits[:-cap]
                    idx = 0
                    while extra:
                        chunk, extra = extra[:2], extra[2:]
                        es = {
                            "name": f"{inst['name']}_ws{idx}",
                            "engine": inst["engine"],
                            "opcode": "EventSemaphore",
                            "ins": [],
                            "outs": [],
                            "sync_info": {"on_wait": chunk, "on_update": []},
                        }
                        if "debug" in inst:
                            es["debug"] = inst["debug"]
                        new_insts.append(es)
                        idx += 1
                    si["on_wait"] = keep
                new_insts.append(inst)
            blk["instructions"] = new_insts
    return json.dumps(m).encode() if changed else bir_bytes


# ---------------------------------------------------------------------------
# Device program (identical on all 8 cores; inputs differ per core)
# ---------------------------------------------------------------------------


def build_nc():
    nc = bass.Bass()

    x_in = nc.dram_tensor("x", [C, T], F32R, kind="ExternalInput")
    wqT_in = nc.dram_tensor("wqT", [C, CH], F32, kind="ExternalInput")
    wkT_in = nc.dram_tensor("wkT", [C, CH], F32, kind="ExternalInput")
    wvT_in = nc.dram_tensor("wvT", [C, CH], F32, kind="ExternalInput")
    bq_in = nc.dram_tensor("bq", [CH, 1], F32, kind="ExternalInput")
    bk_in = nc.dram_tensor("bk", [CH, 1], F32, kind="ExternalInput")
    # proj weights zero-padded to K=128 (rows CH..127 are zero)
    wpT_in = nc.dram_tensor("wpT", [128, C], BF16, kind="ExternalInput")
    oh_in = nc.dram_tensor("oh", [128, 16], F32, kind="ExternalInput")
    ohT_in = nc.dram_tensor("ohT", [16, 128], F32, kind="ExternalInput")
    gnw_in = nc.dram_tensor("gnw", [C, 1], F32, kind="ExternalInput")
    gnb_in = nc.dram_tensor("gnb", [C, 1], F32, kind="ExternalInput")
    out = nc.dram_tensor("out", [C, T], F32, kind="ExternalOutput")
    # softmax denominators (host divides during the gather)
    rsum_out = nc.dram_tensor("rsum", [1, T], BF16, kind="ExternalOutput")
    # GroupNorm b_ch per channel (host folds wv^T b_ch + bv into the gather)
    bch_out = nc.dram_tensor("bch", [2, 128], F32, kind="ExternalOutput")

    with tile.TileContext(nc) as tc:
        with (
            tc.tile_pool(name="const", bufs=1) as const,
            tc.tile_pool(name="xp", bufs=1) as xp,
            tc.tile_pool(name="qk", bufs=1) as qkp,
            tc.tile_pool(name="gn", bufs=2) as gn,
            tc.tile_pool(name="pp", bufs=5) as ppool,
            tc.tile_pool(name="pip", bufs=5) as pip,
            tc.tile_pool(name="op", bufs=4) as op,
            tc.tile_pool(name="ps", bufs=3, space="PSUM") as ps,
            tc.tile_pool(name="ph", bufs=2, space="PSUM") as ph,
        ):
            # ---- x load: 8 column-tiles so bn_stats chases the DMA; issued
            # from the (idle) tensor-engine DMA queue so the small const DMAs
            # on the sync queue don't serialize behind it ----
            x_cols = []  # 8 tiles of [128, 1024]: [half][quarter]
            for i in range(2):
                for quarter in range(4):
                    x_t = xp.tile(
                        [128, 1024], F32R, tag=f"x{i}_{quarter}",
                        name=f"x{i}_{quarter}",
                    )
                    qsl = slice(quarter * 1024, (quarter + 1) * 1024)
                    nc.sync.dma_start(
                        out=x_t, in_=x_in[i * 128 : (i + 1) * 128, qsl]
                    )
                    x_cols.append(x_t)

            def x_ap(ki, lo, hi):
                """view of x[ki*128:(ki+1)*128, lo:hi] (within one col-tile)"""
                t = x_cols[ki * 4 + lo // 1024]
                base = (lo // 1024) * 1024
                return t[:, lo - base : hi - base]

            # ---- load constants/weights (scalar-issued so they don't queue
            # behind the x chunks on the sync engine) ----
            def load_const(name, src, shape, dtype):
                t = const.tile(shape, dtype, tag=name)
                nc.scalar.dma_start(out=t, in_=src[:, :])
                return t

            wq_raw = [
                const.tile([128, CH], F32, tag=f"wqr{i}", name=f"wqr{i}")
                for i in range(2)
            ]
            wk_raw = [
                const.tile([128, CH], F32, tag=f"wkr{i}", name=f"wkr{i}")
                for i in range(2)
            ]
            wv_raw = [
                const.tile([128, CH], F32, tag=f"wvr{i}", name=f"wvr{i}")
                for i in range(2)
            ]
            for i in range(2):
                ksl = slice(i * 128, (i + 1) * 128)
                nc.scalar.dma_start(out=wq_raw[i], in_=wqT_in[ksl, :])
                nc.scalar.dma_start(out=wk_raw[i], in_=wkT_in[ksl, :])
                nc.scalar.dma_start(out=wv_raw[i], in_=wvT_in[ksl, :])

            bq_sb = load_const("bq", bq_in, [CH, 1], F32)
            bk_sb = load_const("bk", bk_in, [CH, 1], F32)
            wpT_sb = load_const("wpT", wpT_in, [128, C], BF16)
            oh_sb = load_const("oh", oh_in, [128, 16], F32)
            ohT_sb = load_const("ohT", ohT_in, [16, 128], F32)

            gnw_t = [
                const.tile([128, 1], F32, tag=f"gnw{i}", name=f"gnw{i}")
                for i in range(2)
            ]
            gnb_t = [
                const.tile([128, 1], F32, tag=f"gnb{i}", name=f"gnb{i}")
                for i in range(2)
            ]
            for i in range(2):
                nc.scalar.dma_start(out=gnw_t[i], in_=gnw_in[i * 128 : (i + 1) * 128, :])
                nc.scalar.dma_start(out=gnb_t[i], in_=gnb_in[i * 128 : (i + 1) * 128, :])

            # preload the Exp table set while the x DMA streams
            warm = gn.tile([1, 1], F32, tag="warm")
            nc.vector.memset(warm, 0.0)
            warm2 = gn.tile([1, 1], F32, tag="warm2")
            nc.scalar.activation(out=warm2, in_=warm, func=AF.Exp)

            # persistent double-buffered hu tiles: rows CH+1..127 zeroed once
            hu_t = []
            for i in range(2):
                t = const.tile([128, TC], BF16, tag=f"hu{i}", name=f"hu{i}")
                nc.gpsimd.memset(t[CH:128, :], 0.0)
                hu_t.append(t)

            # ---- GroupNorm stats -> per-channel a_ch (scale), b_ch (shift).
            # All 16 bn_stats are emitted first (the DVE queue is FIFO, so
            # half-1's stats must not sit behind half-0's reduction chain),
            # then the two halves' chains run interleaved so the per-op
            # semaphore latencies overlap. ----
            stats_t = []
            for i in range(2):
                stats = gn.tile([128, 8, 6], F32, tag=f"stats{i}", name=f"st{i}")
                for j in range(8):
                    xf = x_cols[i * 4 + j // 2].bitcast(F32)
                    wsl = slice((j % 2) * 512, (j % 2) * 512 + 512)
                    nc.vector.bn_stats(out=stats[:, j, :], in_=xf[:, wsl])
                stats_t.append(stats)

            def chain_step(fn):
                for i in range(2):
                    fn(i)

            st = [dict() for _ in range(2)]

            def mk(i, nm, shape=(16, 1), dtype=F32):
                t = gn.tile(list(shape), dtype, tag=f"{nm}{i}", name=f"{nm}{i}")
                st[i][nm] = t
                return t

            chain_step(lambda i: nc.vector.bn_aggr(
                out=mk(i, "mv", (128, 2)), in_=stats_t[i]))
            # mq = [mean, var + mean^2] per channel
            chain_step(lambda i: nc.vector.tensor_copy(
                out=mk(i, "mq", (128, 2))[:, 0:1], in_=st[i]["mv"][:, 0:1]))
            chain_step(lambda i: nc.vector.scalar_tensor_tensor(
                out=st[i]["mq"][:, 1:2], in0=st[i]["mv"][:, 0:1],
                scalar=st[i]["mv"][:, 0:1], in1=st[i]["mv"][:, 1:2],
                op0=ALU.mult, op1=ALU.add))

            # group reduce: [16, 2] = oh.T @ mq   (oh entries are 1/8)
            ps_g = ph.tile([16, 4], F32, tag="ph")
            for i in range(2):
                nc.tensor.matmul(
                    ps_g[:, 2 * i : 2 * i + 2], lhsT=oh_sb, rhs=st[i]["mq"],
                    start=True, stop=True,
                )
            chain_step(lambda i: nc.vector.tensor_copy(
                out=mk(i, "gstats", (16, 2)), in_=ps_g[:, 2 * i : 2 * i + 2]))

            # gve = E[x^2] - mean^2 + eps
            chain_step(lambda i: nc.vector.scalar_tensor_tensor(
                out=mk(i, "gve"), in0=st[i]["gstats"][:, 0:1],
                scalar=st[i]["gstats"][:, 0:1], in1=st[i]["gstats"][:, 1:2],
                op0=ALU.mult, op1=ALU.subtract))
            chain_step(lambda i: nc.vector.tensor_scalar(
                out=st[i]["gve"], in0=st[i]["gve"], scalar1=-1.0, scalar2=EPS,
                op0=ALU.mult, op1=ALU.add))
            # Quake rsqrt + 2 Newton iterations (avoids the Sqrt table set)
            chain_step(lambda i: nc.vector.tensor_scalar(
                out=mk(i, "iv", dtype=I32), in0=st[i]["gve"].bitcast(I32),
                scalar1=1, scalar2=None, op0=ALU.logical_shift_right))
            chain_step(lambda i: nc.vector.tensor_scalar(
                out=st[i]["iv"], in0=st[i]["iv"], scalar1=-1,
                scalar2=0x5F3759DF, op0=ALU.mult, op1=ALU.add))
            chain_step(lambda i: nc.vector.tensor_scalar(
                out=mk(i, "hv"), in0=st[i]["gve"], scalar1=0.5, scalar2=None,
                op0=ALU.mult))
            for i in range(2):
                st[i]["cur"] = st[i]["iv"].bitcast(F32)
                mk(i, "yy")
                mk(i, "t2")
                mk(i, "grstd")
            for it in range(2):
                chain_step(lambda i: nc.vector.tensor_tensor(
                    out=st[i]["yy"], in0=st[i]["cur"], in1=st[i]["cur"],
                    op=ALU.mult))
                chain_step(lambda i: nc.vector.tensor_tensor(
                    out=st[i]["yy"], in0=st[i]["yy"], in1=st[i]["hv"],
                    op=ALU.mult))
                chain_step(lambda i: nc.vector.tensor_scalar(
                    out=st[i]["yy"], in0=st[i]["yy"], scalar1=-1.0, scalar2=1.5,
                    op0=ALU.mult, op1=ALU.add))

                def newt(i, it=it):
                    dst = st[i]["grstd"] if it == 1 else st[i]["t2"]
                    nc.vector.tensor_tensor(
                        out=dst, in0=st[i]["cur"], in1=st[i]["yy"], op=ALU.mult
                    )
                    st[i]["cur"] = dst
                chain_step(newt)

            chain_step(lambda i: nc.vector.tensor_copy(
                out=mk(i, "gmr", (16, 2))[:, 0:1], in_=st[i]["gstats"][:, 0:1]))
            chain_step(lambda i: nc.vector.tensor_copy(
                out=st[i]["gmr"][:, 1:2], in_=st[i]["grstd"]))

            # broadcast back to channels: [128, 2] = ohT.T @ gmr
            ps_bc = ph.tile([128, 4], F32, tag="ph")
            for i in range(2):
                nc.tensor.matmul(
                    ps_bc[:, 2 * i : 2 * i + 2], lhsT=ohT_sb, rhs=st[i]["gmr"],
                    start=True, stop=True,
                )
            chain_step(lambda i: nc.vector.tensor_tensor(
                out=mk(i, "a_ch", (128, 1)), in0=ps_bc[:, 2 * i + 1 : 2 * i + 2],
                in1=gnw_t[i], op=ALU.mult))
            chain_step(lambda i: nc.vector.tensor_tensor(
                out=mk(i, "t1", (128, 1)), in0=ps_bc[:, 2 * i : 2 * i + 1],
                in1=st[i]["a_ch"], op=ALU.mult))
            chain_step(lambda i: nc.vector.tensor_tensor(
                out=mk(i, "b_ch", (128, 1)), in0=gnb_t[i], in1=st[i]["t1"],
                op=ALU.subtract))
            for i in range(2):
                nc.sync.dma_start(out=bch_out[i : i + 1, :], in_=st[i]["b_ch"])
            a_ch = [(st[i]["a_ch"], st[i]["b_ch"]) for i in range(2)]

            # ---- fold GN into the qkv weights/biases ----
            wqf = [
                qkp.tile([128, CH], F32R, tag=f"wqf{i}", name=f"wqf{i}")
                for i in range(2)
            ]
            wkf = [
                qkp.tile([128, CH], F32R, tag=f"wkf{i}", name=f"wkf{i}")
                for i in range(2)
            ]
            wvf = [
                qkp.tile([128, CH], F32R, tag=f"wvf{i}", name=f"wvf{i}")
                for i in range(2)
            ]
            for i in range(2):
                nc.scalar.activation(
                    out=wqf[i], in_=wq_raw[i], func=AF.Identity, scale=a_ch[i][0]
                )
                nc.scalar.activation(
                    out=wkf[i], in_=wk_raw[i], func=AF.Identity, scale=a_ch[i][0]
                )
                nc.scalar.activation(
                    out=wvf[i], in_=wv_raw[i], func=AF.Identity, scale=a_ch[i][0]
                )

            # bias' = b + wX_raw^T b_ch  (tiny N=1 matmuls, f32)
            ps_bias = ph.tile([CH, 2], F32, tag="ph")
            for col, wr in ((0, wq_raw), (1, wk_raw)):
                for ki in range(2):
                    nc.tensor.matmul(
                        ps_bias[:, col : col + 1],
                        lhsT=wr[ki],
                        rhs=a_ch[ki][1],
                        start=(ki == 0),
                        stop=(ki == 1),
                    )
            bqf = gn.tile([CH, 1], F32, tag="bqf")
            bkf = gn.tile([CH, 1], F32, tag="bkf")
            nc.vector.tensor_tensor(
                out=bqf, in0=ps_bias[:, 0:1], in1=bq_sb, op=ALU.add
            )
            nc.vector.tensor_tensor(
                out=bkf, in0=ps_bias[:, 1:2], in1=bk_sb, op=ALU.add
            )

            # ---- q/k/v targets ----
            # q, k zero-padded to K=128: the full-array contraction keeps the
            # HAM clock gate warm at 2.4 GHz (K=64 variants run cold/serial)
            q_sb = qkp.tile([128, T], BF16, tag="q")
            k_sb = qkp.tile([128, T], BF16, tag="k")
            nc.gpsimd.memset(q_sb[CH:128, :], 0.0)
            nc.gpsimd.memset(k_sb[CH:128, :], 0.0)
            vT = qkp.tile([128, N_SBLK * 80], FP8, tag="vT")
            nc.gpsimd.memset(vT, 1.0)
            vT_view = vT.rearrange("p (b c) -> p b c", c=80)

            ring = [0]  # rotation counter (pool handles the actual slots)

            def ring_slice():
                r = ring[0] % 3
                ring[0] += 1
                return r, ps.tile([128, 1024], F32, tag="ps", name="ps_s")

            def emit_qk_chunk(dst, w, bias, n):
                _, psq = ring_slice()
                psq = psq[0:CH, :]
                for nj in range(2):
                    sl = slice(nj * 512, (nj + 1) * 512)
                    lo = n * 1024 + nj * 512
                    for ki in range(2):
                        nc.tensor.matmul(
                            psq[:, sl],
                            lhsT=w[ki],
                            rhs=x_ap(ki, lo, lo + 512),
                            start=(ki == 0),
                            stop=(ki == 1),
                        )
                with nc.allow_low_precision(reason="bf16 q/k"):
                    nc.scalar.activation(
                        out=dst[0:CH, n * 1024 : (n + 1) * 1024],
                        in_=psq,
                        func=AF.Identity,
                        bias=bias,
                    )

            def emit_vt_chunk(pblk):
                psv = ph.tile([128, 512], F32, tag="ph", name=f"psv{pblk}")
                for j in range(8):
                    sblk = pblk * 8 + j
                    sl = slice(j * 64, (j + 1) * 64)
                    for ki in range(2):
                        nc.tensor.matmul(
                            psv[:, sl],
                            lhsT=x_ap(ki, sblk * 128, (sblk + 1) * 128),
                            rhs=wvf[ki],
                            start=(ki == 0),
                            stop=(ki == 1),
                        )
                with nc.allow_low_precision(reason="fp8 v"):
                    nc.scalar.copy(
                        out=vT_view[:, pblk * 8 : (pblk + 1) * 8, 0:64],
                        in_=psv.rearrange("p (b c) -> p b c", c=64),
                    )

            def emit_qkv_step(c):
                emit_qk_chunk(k_sb, wkf, bkf, c)
                if c == 0:
                    emit_qk_chunk(q_sb, wqf, bqf, 0)

            emit_qkv_step(0)

            # ---- attention + proj, streamed per t-chunk ----
            PV_LAG = 3  # pv(j) is emitted
            # after scores(j+PV_LAG) so the in-order PE queue never stalls
            # waiting for exp(j)

            # greedy engine balance for exp work (ns accounting); t-chunk 0
            # pre-charges the scalar engine for the q/k/v PSUM->SBUF copies
            ebal = {"S": 0.0, "V": 0.0}

            def exp_engine(width):
                cost_s = 352 + width
                cost_v = (120 + width) * 1.25
                if ebal["S"] + cost_s <= ebal["V"] + cost_v:
                    ebal["S"] += cost_s
                    return "S"
                ebal["V"] += cost_v
                return "V"

            def emit_exp(slots):
                """slots: list of (j, ps_s tile), len 1 (no cross-tile merge)."""
                assert len(slots) == 1
                width = 1024
                src = slots[0][1]
                eng = exp_engine(width)
                if eng == "V":
                    pi_t = pip.tile([128, width], I8, tag=f"pi{len(slots)}",
                                    name="pi")
                    with nc.allow_low_precision(reason="fast exp"):
                        nc.vector.tensor_scalar(
                            out=pi_t, in0=src, scalar1=A8, scalar2=B8,
                            op0=ALU.mult, op1=ALU.add,
                        )
                    pb = pi_t.bitcast(FP8)
                else:
                    pb = ppool.tile([128, width], FP8, tag=f"p{len(slots)}",
                                    name="p")
                    with nc.allow_low_precision(reason="fp8 p"):
                        nc.scalar.activation(
                            out=pb, in_=src, func=AF.Exp, scale=EXP_SC,
                        )
                return [
                    (j, pb[:, i * 1024 : (i + 1) * 1024])
                    for i, (j, _) in enumerate(slots)
                ]

            T0_CHARGE = 11000.0  # tchunk-0 scalar precharge (q/k/v copies)

            def body(tci):
                tsl = slice(tci * TC, (tci + 1) * TC)
                if tci == 0:
                    ebal["S"] += T0_CHARGE
                ps_h = ph.tile([65, TC], F32, tag="ph", name=f"ps_h{tci}")
                pending = []  # pv operands awaiting emission
                unexp = []  # score slots awaiting exp

                def emit_pv(j, pb):
                    pr = pb.rearrange("p (b c) -> p b c", c=TC)
                    nc.tensor.matmul(
                        ps_h,
                        lhsT=vT_view[:, 2 * j : 2 * j + 2, 0:65],
                        rhs=pr[:, 0:2, :],
                        start=(j == 0),
                        stop=(j == N_PAIR - 1),
                        perf_mode=mybir.MatmulPerfMode.DoubleRow,
                    )

                def flush_exp():
                    if unexp:
                        pending.extend(emit_exp(unexp))
                        unexp.clear()

                for j in range(N_PAIR):
                    if tci == 0 and j in (4, 8, 12):
                        flush_exp()
                        emit_qkv_step(j // 4)
                        if j == 4:
                            for qc in range(1, 4):
                                emit_qk_chunk(q_sb, wqf, bqf, qc)
                    if tci == 0 and j in (1, 5, 9, 13):
                        # v chunks trail the k/q chunks so the first scores
                        # (and HAM warm-up) start as early as possible
                        emit_vt_chunk(j // 4)
                    r, ps_s = ring_slice()
                    for jj in range(2):
                        sblk = 2 * j + jj
                        nc.tensor.matmul(
                            ps_s[:, jj * 512 : (jj + 1) * 512],
                            lhsT=k_sb[:, sblk * 128 : (sblk + 1) * 128],
                            rhs=q_sb[:, tsl],
                            start=True,
                            stop=True,
                        )
                    unexp.append((j, ps_s))
                    flush_exp()
                    while len(pending) > PV_LAG:
                        emit_pv(*pending.pop(0))
                flush_exp()
                for item in pending:
                    emit_pv(*item)
                return ps_h

            def epilogue(tci, ps_h):
                tsl = slice(tci * TC, (tci + 1) * TC)
                # ship the softmax denominators; the division commutes with
                # the proj channel-contraction and the host's gather applies it
                hu = hu_t[tci % 2]
                with nc.allow_low_precision(reason="bf16 h"):
                    nc.vector.tensor_copy(out=hu[0:65, :], in_=ps_h[0:65, :])
                nc.sync.dma_start(out=rsum_out[0:1, tsl], in_=hu[64:65, :])
                for mi in range(2):
                    pp_ps = ph.tile([128, TC], F32, tag="ph", name=f"pp{tci}_{mi}")
                    nc.tensor.matmul(
                        pp_ps,
                        lhsT=wpT_sb[:, mi * 128 : (mi + 1) * 128],
                        rhs=hu,
                        start=True,
                        stop=True,
                    )
                    o_t = op.tile([128, TC], F32, tag="o", name=f"o{tci}_{mi}")
                    if mi == 0:
                        nc.scalar.copy(out=o_t, in_=pp_ps)
                    else:
                        nc.vector.tensor_copy(out=o_t, in_=pp_ps)
                    nc.sync.dma_start(
                        out=out[mi * 128 : (mi + 1) * 128, tsl], in_=o_t
                    )

            # software pipeline: emit chunk i+1's matmuls before chunk i's
            # epilogue so the PE never stalls on the epilogue path
            prev = None
            for tci in range(N_TCHUNKS):
                ps_h = body(tci)
                if prev is not None:
                    epilogue(tci - 1, prev)
                prev = ps_h
            epilogue(N_TCHUNKS - 1, prev)

    # wrap to_json_bytes with the wait legalization
    orig = nc.to_json_bytes
    nc.to_json_bytes = lambda *a, **k: _legalize_bir_waits(orig(*a, **k))
    return nc


_NC = None


def _get_nc():
    global _NC
    if _NC is None:
        _NC = build_nc()
    return _NC


def _make_in_maps(inputs):
    x = np.asarray(inputs["x"], dtype=np.float32)
    gn_w = np.asarray(inputs["gn_w"], dtype=np.float32)
    gn_b = np.asarray(inputs["gn_b"], dtype=np.float32)
    qkv_w = np.asarray(inputs["qkv_w"], dtype=np.float32)
    qkv_b = np.asarray(inputs["qkv_b"], dtype=np.float32)
    proj_w = np.asarray(inputs["proj_w"], dtype=np.float32)

    xs = x.reshape(B, C, T)
    oh = np.kron(np.eye(16, dtype=np.float32), np.full((8, 1), 0.125, np.float32))
    ohT = np.ascontiguousarray(oh.T) * 8.0  # plain one-hot [16, 128]
    gnw = gn_w.reshape(C, 1)
    gnb = gn_b.reshape(C, 1)

    in_maps = []
    for core in range(N_CORES):
        b, h = divmod(core, NH)
        # reference reshapes (b, 3c, T) -> (b*nh, 3*ch, T) then splits dim 1,
        # so head h takes qkv rows [3*ch*h : 3*ch*(h+1)] as [q | k | v]
        base = 3 * CH * h
        qsl = slice(base, base + CH)
        ksl = slice(base + CH, base + 2 * CH)
        vsl = slice(base + 2 * CH, base + 3 * CH)
        wqT = np.ascontiguousarray(qkv_w[qsl, :].T).astype(np.float32)
        wkT = np.ascontiguousarray(qkv_w[ksl, :].T).astype(np.float32)
        wvT = np.ascontiguousarray(qkv_w[vsl, :].T).astype(np.float32)
        bq = qkv_b[qsl].reshape(CH, 1).astype(np.float32)
        bk = qkv_b[ksl].reshape(CH, 1).astype(np.float32)
        # after attention, h.reshape(b, c, T) stacks heads along channels:
        # head h occupies channels [ch*h : ch*(h+1)]; padded to K=128 rows
        wpT = np.zeros((128, C), ml_dtypes.bfloat16)
        wpT[0:CH] = proj_w[:, h * CH : (h + 1) * CH].T.astype(ml_dtypes.bfloat16)
        in_maps.append(
            {
                "x": np.ascontiguousarray(xs[b]),
                "wqT": wqT,
                "wkT": wkT,
                "wvT": wvT,
                "bq": bq,
                "bk": bk,
                "wpT": wpT,
                "oh": oh,
                "ohT": ohT,
                "gnw": gnw,
                "gnb": gnb,
            }
        )
    return in_maps


def _combine(inputs, results):
    x = np.asarray(inputs["x"], dtype=np.float32)
    proj_b = np.asarray(inputs["proj_b"], dtype=np.float32)
    qkv_b = np.asarray(inputs["qkv_b"], dtype=np.float32)
    qkv_w = np.asarray(inputs["qkv_w"], dtype=np.float32)
    proj_w = np.asarray(inputs["proj_w"], dtype=np.float32)
    xs = x.reshape(B, C, T)
    out = np.empty((B, C, T), np.float32)
    for b in range(B):
        acc = xs[b] + proj_b[:, None]
        b_ch = results[b * NH]["bch"].reshape(C)
        for h in range(NH):
            r = results[b * NH + h]
            # v's effective bias (wv^T b_ch + bv) contributes a constant per
            # channel to h after the rowsum division; it commutes with proj
            vsl = slice(3 * CH * h + 2 * CH, 3 * CH * (h + 1))
            bv_eff = qkv_w[vsl, :] @ b_ch + qkv_b[vsl]
            wpbv = proj_w[:, h * CH : (h + 1) * CH] @ bv_eff
            acc = (
                acc
                + r["out"] * (1.0 / r["rsum"][0].astype(np.float32))[None, :]
                + wpbv[:, None]
            )
        out[b] = acc
    return out.reshape(B, C, HW, HW)


def _run(inputs, trace=False, trace_kwargs=None):
    nc = _get_nc()
    in_maps = _make_in_maps(inputs)
    res = run_bass_kernel_spmd(
        nc,
        in_maps,
        core_ids=list(range(N_CORES)),
        trace=trace,
        **(trace_kwargs or {}),
    )
    return _combine(inputs, res.results), res


def kernel(**inputs) -> np.ndarray:
    out, _ = _run(inputs, trace=False)
    return out
